# revision 9
# baseline (speedup 1.0000x reference)
"""Disentangled self-attention (DeBERTa-style) Trainium2 kernel, 8 NeuronCores.

Math restructuring: the reference projects pos_emb (S,S,H) through Wpk/Wpq
(~348 GFLOP).  Because each c2p/p2c score element only contracts the projected
vector with q/k, we instead contract q/k with the weight slices first:

    c2p[h,i,j] = sum_c qpk[h,i,c] * pos[i,j,c]   (+ q.bpk_h, const over j ->
                                                  cancels in softmax)
    p2c[h,i,j] = sum_c kpq[h,j,c] * pos[j,i,c]   + k[j].bpq_h
    qpk[h,i,c] = sum_d Wpk[c,hD+d] q[i,hD+d],  kpq likewise with Wpq/k

which drops the pos-side work to ~6 GFLOP and makes the single read of
pos_emb the bottleneck.

Sharding: core c owns slab t in [48c, 48c+48).  The slab pos[t,:,:] serves
both c2p rows i=t and p2c columns j=t (same (384,768) slice, second index
means j resp. i).  p2c is produced column-sharded and moved to the row owners
with one small AllToAll (8 x 12x48x48), then re-transposed through the PE.
Everything else (projections, c2c, softmax, probs@v) is computed on the row
owner.  pos is cast to bf16 on the host so DMA-transpose (2-byte only) can
deliver it H-major (contraction dim on partitions) straight from HBM.
"""

import sys

sys.path.insert(0, "/opt/trn_rl_repo")

import math
import numpy as np
import ml_dtypes

import concourse.bass as bass
import concourse.bacc as bacc
import concourse.mybir as mybir
import concourse.tile as tile
from concourse.bass_utils import run_bass_kernel_spmd

BF16 = mybir.dt.bfloat16
F32 = mybir.dt.float32
AF = mybir.ActivationFunctionType

S = 384
H = 768
NH = 12
D = 64
NC = 8
TB = S // NC  # 48 rows per core
NCH = H // 128  # 6 chunks of the hidden dim


def build_module():
    nc = bacc.Bacc(trn_type="TRN2", num_devices=NC, debug=False)

    # ---- I/O ----
    pos_d = nc.dram_tensor("pos", [TB, H, S], BF16, kind="ExternalInput")
    hsT_d = nc.dram_tensor("hsT", [H, S], BF16, kind="ExternalInput")
    hsTo_d = nc.dram_tensor("hsTo", [H, TB], BF16, kind="ExternalInput")
    wq_d = nc.dram_tensor("wq", [H, H], BF16, kind="ExternalInput")
    wk_d = nc.dram_tensor("wk", [H, H], BF16, kind="ExternalInput")
    wv_d = nc.dram_tensor("wv", [H, H], BF16, kind="ExternalInput")
    wpkT_d = nc.dram_tensor("wpkT", [H, H], BF16, kind="ExternalInput")
    wpqT_d = nc.dram_tensor("wpqT", [H, H], BF16, kind="ExternalInput")
    bqT_d = nc.dram_tensor("bqT", [128, NCH], F32, kind="ExternalInput")
    bkT_d = nc.dram_tensor("bkT", [128, NCH], F32, kind="ExternalInput")
    bv_d = nc.dram_tensor("bv", [H], F32, kind="ExternalInput")
    bpqd_d = nc.dram_tensor("bpqd", [128, NCH, NH], BF16, kind="ExternalInput")
    mask_d = nc.dram_tensor("maskrow", [S], F32, kind="ExternalInput")
    ident_d = nc.dram_tensor("ident", [128, 128], BF16, kind="ExternalInput")
    out_d = nc.dram_tensor("out", [TB, H], F32, kind="ExternalOutput")

    with tile.TileContext(nc) as tc:
        with (
            tc.tile_pool(name="const", bufs=1) as cpool,
            tc.tile_pool(name="work", bufs=1) as wpool,
            tc.tile_pool(name="posT", bufs=3) as ppool,
            tc.tile_pool(name="stg", bufs=4) as spool,
            tc.tile_pool(name="psum", bufs=8, space="PSUM") as pspool,
            tc.tile_pool(name="dram", bufs=1, space="DRAM") as dpool,
        ):
            # ---- load constants ----
            hsT = [cpool.tile([128, S], BF16, tag=f"hsT{m}", name=f"hsT{m}") for m in range(NCH)]
            hsTo = [cpool.tile([128, TB], BF16, tag=f"hsTo{m}", name=f"hsTo{m}") for m in range(NCH)]
            wq = [cpool.tile([128, H], BF16, tag=f"wq{m}", name=f"wq{m}") for m in range(NCH)]
            wk = [cpool.tile([128, H], BF16, tag=f"wk{m}", name=f"wk{m}") for m in range(NCH)]
            wv = [cpool.tile([128, H], BF16, tag=f"wv{m}", name=f"wv{m}") for m in range(NCH)]
            wpkT = [cpool.tile([128, H], BF16, tag=f"wpkT{m}", name=f"wpkT{m}") for m in range(NCH)]
            wpqT = [cpool.tile([128, H], BF16, tag=f"wpqT{m}", name=f"wpqT{m}") for m in range(NCH)]
            for m in range(NCH):
                r = slice(m * 128, (m + 1) * 128)
                nc.gpsimd.dma_start(hsT[m][:], hsT_d[r, :])
                nc.gpsimd.dma_start(hsTo[m][:], hsTo_d[r, :])
                nc.gpsimd.dma_start(wq[m][:], wq_d[r, :])
                nc.gpsimd.dma_start(wk[m][:], wk_d[r, :])
                nc.gpsimd.dma_start(wv[m][:], wv_d[r, :])
                nc.gpsimd.dma_start(wpkT[m][:], wpkT_d[r, :])
                nc.gpsimd.dma_start(wpqT[m][:], wpqT_d[r, :])
            bqT = cpool.tile([128, NCH], F32, tag="bqT")
            bkT = cpool.tile([128, NCH], F32, tag="bkT")
            bpqd = cpool.tile([128, NCH, NH], BF16, tag="bpqd")
            ident = cpool.tile([128, 128], BF16, tag="ident")
            nc.gpsimd.dma_start(bqT[:], bqT_d[:])
            nc.gpsimd.dma_start(bkT[:], bkT_d[:])
            nc.gpsimd.dma_start(bpqd[:], bpqd_d[:])
            nc.gpsimd.dma_start(ident[:], ident_d[:])
            bvbc = cpool.tile([128, H], BF16, tag="bvbc")
            nc.gpsimd.dma_start(bvbc[:], bv_d[:].partition_broadcast(128))
            mask12 = cpool.tile([NH, S], F32, tag="mask12")
            nc.gpsimd.dma_start(mask12[:], mask_d[:].partition_broadcast(NH))

            # ---- projections ----
            # kT[hd, j] (full), qT_own[hd, i own], kT_own[hd, j own], v[j, hd]
            kT = [wpool.tile([128, S], BF16, tag=f"kT{m}", name=f"kT{m}") for m in range(NCH)]
            qTo = [wpool.tile([128, TB], BF16, tag=f"qTo{m}", name=f"qTo{m}") for m in range(NCH)]
            kTo = [wpool.tile([128, TB], BF16, tag=f"kTo{m}", name=f"kTo{m}") for m in range(NCH)]
            for m in range(NCH):
                ps = pspool.tile([128, S], F32, tag="ps")
                for c in range(NCH):
                    nc.tensor.matmul(
                        ps[:], wk[c][:, m * 128 : (m + 1) * 128], hsT[c][:],
                        start=(c == 0), stop=(c == NCH - 1),
                    )
                nc.vector.tensor_scalar_add(kT[m][:], ps[:], bkT[:, m : m + 1])
                pso = pspool.tile([128, TB], F32, tag="ps")
                for c in range(NCH):
                    nc.tensor.matmul(
                        pso[:], wq[c][:, m * 128 : (m + 1) * 128], hsTo[c][:],
                        start=(c == 0), stop=(c == NCH - 1),
                    )
                nc.vector.tensor_scalar_add(qTo[m][:], pso[:], bqT[:, m : m + 1])
                psk = pspool.tile([128, TB], F32, tag="ps")
                for c in range(NCH):
                    nc.tensor.matmul(
                        psk[:], wk[c][:, m * 128 : (m + 1) * 128], hsTo[c][:],
                        start=(c == 0), stop=(c == NCH - 1),
                    )
                nc.vector.tensor_scalar_add(kTo[m][:], psk[:], bkT[:, m : m + 1])

            v_sb = [wpool.tile([128, H], BF16, tag=f"v{jc}", name=f"v{jc}") for jc in range(3)]
            for jc in range(3):
                for nh in range(2):
                    ps = pspool.tile([128, S], F32, tag="ps")
                    for c in range(NCH):
                        nc.tensor.matmul(
                            ps[:],
                            hsT[c][:, jc * 128 : (jc + 1) * 128],
                            wv[c][:, nh * S : (nh + 1) * S],
                            start=(c == 0), stop=(c == NCH - 1),
                        )
                    nc.scalar.activation(v_sb[jc][:, nh * S : (nh + 1) * S], ps[:], AF.Copy)
                nc.vector.tensor_tensor(
                    v_sb[jc][:], v_sb[jc][:], bvbc[:], op=mybir.AluOpType.add
                )

            # ---- qkp[c_chunk][128, t, 24]: cols 0:12 qpk (Wpk.T q), 12:24 kpq ----
            qkp = [wpool.tile([128, TB, 2 * NH], BF16, tag=f"qkp{m}", name=f"qkp{m}") for m in range(NCH)]
            for h in range(NH):
                mh, oh = h // 2, (h % 2) * 64
                for m in range(NCH):
                    ps = pspool.tile([128, TB], F32, tag="ps")
                    nc.tensor.matmul(
                        ps[:],
                        wpkT[mh][oh : oh + 64, m * 128 : (m + 1) * 128],
                        qTo[mh][oh : oh + 64, :],
                    )
                    nc.scalar.activation(qkp[m][:, :, h], ps[:], AF.Copy)
                    ps2 = pspool.tile([128, TB], F32, tag="ps")
                    nc.tensor.matmul(
                        ps2[:],
                        wpqT[mh][oh : oh + 64, m * 128 : (m + 1) * 128],
                        kTo[mh][oh : oh + 64, :],
                    )
                    nc.scalar.activation(qkp[m][:, :, NH + h], ps2[:], AF.Copy)

            # ---- colbias[h, j] = (k[j] . bpq_h) / sqrt(D) + mask[j] ----
            pskb = pspool.tile([NH, S], F32, tag="ps")
            for m in range(NCH):
                nc.tensor.matmul(
                    pskb[:], bpqd[:, m, :], kT[m][:],
                    start=(m == 0), stop=(m == NCH - 1),
                )
            colbias = wpool.tile([NH, S], F32, tag="colbias")
            nc.scalar.activation(colbias[:], pskb[:], AF.Copy, scale=1.0 / math.sqrt(D))
            nc.vector.tensor_tensor(
                colbias[:], colbias[:], mask12[:], op=mybir.AluOpType.add
            )
            cb_dram = dpool.tile([NH, S], F32)
            nc.gpsimd.dma_start(cb_dram[:], colbias[:])
            colbias_bc = wpool.tile([TB, NH, S], F32, tag="colbias_bc")
            nc.gpsimd.dma_start(colbias_bc[:], cb_dram[:].partition_broadcast(TB))

            # ---- main loop over own slabs t: c2p rows + p2c cols ----
            c2p_dram = dpool.tile([TB, NH, S], BF16)
            a2a_in = dpool.tile([NC, NH, TB, TB], BF16)  # [dest, h, t, i_local]
            a2a_out = dpool.tile([NC, NH, TB, TB], BF16)  # [src, h, t', i_local]
            for t in range(TB):
                posT = [
                    ppool.tile([128, S], BF16, tag=f"posT{m}", name=f"posT{m}")
                    for m in range(NCH)
                ]
                for m in range(NCH):
                    nc.sync.dma_start(posT[m][:], pos_d[t, m * 128 : (m + 1) * 128, :])
                ps = pspool.tile([2 * NH, S], F32, tag="ps")
                for m in range(NCH):
                    nc.tensor.matmul(
                        ps[:], qkp[m][:, t, :], posT[m][:],
                        start=(m == 0), stop=(m == NCH - 1),
                    )
                stage = spool.tile([2 * NH, S], BF16, tag="stage", name="stage")
                nc.scalar.activation(stage[:], ps[:], AF.Copy)
                nc.gpsimd.dma_start(c2p_dram[t], stage[0:NH, :])
                nc.gpsimd.dma_start(
                    a2a_in[:, :, t, :].rearrange("d h i -> h d i"),
                    stage[NH : 2 * NH, :].rearrange("h (d i) -> h d i", d=NC),
                )

            # ---- c2c rows ----
            scores = wpool.tile([TB, NH, S], F32, tag="scores")
            for h in range(NH):
                mh, oh = h // 2, (h % 2) * 64
                ps = pspool.tile([TB, S], F32, tag="ps")
                nc.tensor.matmul(ps[:], qTo[mh][oh : oh + 64, :], kT[mh][oh : oh + 64, :])
                nc.scalar.activation(scores[:, h, :], ps[:], AF.Copy)

            # ---- exchange p2c columns -> rows ----
            nc.gpsimd.collective_compute(
                "AllToAll",
                mybir.AluOpType.bypass,
                replica_groups=[list(range(NC))],
                ins=[a2a_in.opt()],
                outs=[a2a_out.opt()],
            )

            # ---- p2c rows: transpose a2a_out [(s h t), il] -> [il, (s h t)] ----
            p2c_rows = wpool.tile([TB, NC * NH * TB], BF16, tag="p2c_rows")
            g2flat = a2a_out[:].rearrange("s h t i -> (s h t) i")
            NT = NC * NH * TB // 128  # 36 tiles of 128 rows
            for m in range(NT):
                g2t = spool.tile([128, TB], BF16, tag="g2t", name="g2t")
                nc.sync.dma_start(g2t[:], g2flat[m * 128 : (m + 1) * 128, :])
                pst = pspool.tile([TB, 128], BF16, tag="ps")
                nc.tensor.transpose(pst[:], g2t[:], ident[:])
                nc.vector.tensor_copy(p2c_rows[:, m * 128 : (m + 1) * 128], pst[:])

            # ---- assemble scores rows, softmax ----
            c2p_rows = wpool.tile([TB, NH, S], BF16, tag="c2p_rows")
            nc.sync.dma_start(c2p_rows[:], c2p_dram[:])

            nc.vector.tensor_tensor(
                scores[:], scores[:], c2p_rows[:], op=mybir.AluOpType.add
            )
            sc4 = scores[:].rearrange("i h (s t) -> i h s t", s=NC)
            nc.vector.tensor_tensor(
                sc4,
                sc4,
                p2c_rows[:].rearrange("i (s h t) -> i h s t", s=NC, h=NH),
                op=mybir.AluOpType.add,
            )
            # scores/sqrt(D) + colbias  (colbias = kb/sqrt(D) + mask)
            nc.scalar.activation(scores[:], scores[:], AF.Copy, scale=1.0 / math.sqrt(D))
            nc.vector.tensor_tensor(
                scores[:], scores[:], colbias_bc[:], op=mybir.AluOpType.add
            )

            negmax = wpool.tile([TB, NH], F32, tag="negmax")
            sums = wpool.tile([TB, NH], F32, tag="sums")
            recip = wpool.tile([TB, NH], F32, tag="recip")
            probs = wpool.tile([TB, NH, S], BF16, tag="probs")
            for h in range(NH):
                nc.vector.tensor_reduce(
                    out=negmax[:, h : h + 1], in_=scores[:, h, :],
                    op=mybir.AluOpType.max, axis=mybir.AxisListType.X, negate=True,
                )
                nc.scalar.activation(
                    probs[:, h, :], scores[:, h, :], AF.Exp,
                    bias=negmax[:, h : h + 1],
                    accum_out=sums[:, h : h + 1],
                )
            nc.vector.reciprocal(recip[:], sums[:])

            # ---- probs.T per head, then ctx = probs @ v ----
            ptile = [wpool.tile([128, NH, TB], BF16, tag=f"pt{jc}", name=f"pt{jc}") for jc in range(3)]
            for h in range(NH):
                for jc in range(3):
                    pst = pspool.tile([128, TB], BF16, tag="ps")
                    nc.tensor.transpose(
                        pst[:], probs[:, h, jc * 128 : (jc + 1) * 128], ident[0:TB, 0:TB]
                    )
                    nc.vector.tensor_copy(ptile[jc][:, h, :], pst[:])
            out_sb = wpool.tile([TB, H], F32, tag="out_sb")
            for h in range(NH):
                psc = pspool.tile([TB, D], F32, tag="ps")
                for jc in range(3):
                    nc.tensor.matmul(
                        psc[:], ptile[jc][:, h, :], v_sb[jc][:, h * D : (h + 1) * D],
                        start=(jc == 0), stop=(jc == 2),
                    )
                nc.vector.tensor_scalar_mul(
                    out_sb[:, h * D : (h + 1) * D], psc[:], recip[:, h : h + 1]
                )
            nc.sync.dma_start(out_d[:], out_sb[:])

    nc.compile()
    return nc


_NC_CACHE = None


def _prep_inputs(hidden_states, attention_mask, pos_emb, Wq, bq, Wk, bk, Wv, bv,
                 Wpk, bpk, Wpq, bpq):
    bf = ml_dtypes.bfloat16
    hs = np.ascontiguousarray(np.asarray(hidden_states, np.float32)[0])  # (S, H)
    hsT = np.ascontiguousarray(hs.T).astype(bf)
    wq = np.asarray(Wq, np.float32).astype(bf)
    wk = np.asarray(Wk, np.float32).astype(bf)
    wv = np.asarray(Wv, np.float32).astype(bf)
    wpkT = np.ascontiguousarray(np.asarray(Wpk, np.float32).T).astype(bf)
    wpqT = np.ascontiguousarray(np.asarray(Wpq, np.float32).T).astype(bf)
    bqT = np.ascontiguousarray(np.asarray(bq, np.float32).reshape(NCH, 128).T)
    bkT = np.ascontiguousarray(np.asarray(bk, np.float32).reshape(NCH, 128).T)
    bpq_f = np.asarray(bpq, np.float32)
    bpqd = np.zeros((128, NCH, NH), bf)
    for m in range(NCH):
        for half in range(2):
            h = 2 * m + half
            bpqd[64 * half : 64 * half + 64, m, h] = bpq_f[
                128 * m + 64 * half : 128 * m + 64 * half + 64
            ].astype(bf)
    mask_row = np.ascontiguousarray(np.asarray(attention_mask, np.float32)[0, 0, 0])
    ident = np.eye(128, dtype=bf)

    common = dict(
        hsT=hsT, wq=wq, wk=wk, wv=wv, wpkT=wpkT, wpqT=wpqT,
        bqT=bqT, bkT=bkT, bv=np.asarray(bv, np.float32),
        bpqd=bpqd, maskrow=mask_row, ident=ident,
    )
    in_maps = []
    pos0 = np.asarray(pos_emb)[0]  # (S, S, H) f32
    for c in range(NC):
        sl = slice(c * TB, (c + 1) * TB)
        m = dict(common)
        m["pos"] = pos0[sl].transpose(0, 2, 1).astype(bf)
        m["hsTo"] = np.ascontiguousarray(hsT[:, sl])
        in_maps.append(m)
    return in_maps


def kernel(**inputs):
    global _NC_CACHE
    if _NC_CACHE is None:
        _NC_CACHE = build_module()
    nc = _NC_CACHE
    in_maps = _prep_inputs(**inputs)
    res = run_bass_kernel_spmd(nc, in_maps, core_ids=list(range(NC)))
    out = np.concatenate([r["out"] for r in res.results], axis=0)
    return out.reshape(1, S, H).astype(np.float32)


# revision 12
# speedup vs baseline: 1.3620x; 1.3620x over previous
"""Disentangled self-attention (DeBERTa-style) Trainium2 kernel, 8 NeuronCores.

Math restructuring: the reference projects pos_emb (S,S,H) through Wpk/Wpq
(~348 GFLOP).  Because each c2p/p2c score element only contracts the projected
vector with q/k, we instead contract q/k with the weight slices first:

    c2p[h,i,j] = sum_c qpk[h,i,c] * pos[i,j,c]   (+ q.bpk_h, const over j ->
                                                  cancels in softmax)
    p2c[h,i,j] = sum_c kpq[h,j,c] * pos[j,i,c]   + k[j].bpq_h
    qpk[h,i,c] = sum_d Wpk[c,hD+d] q[i,hD+d],  kpq likewise with Wpq/k

which drops the pos-side work to ~6 GFLOP and makes the single read of
pos_emb the bottleneck.

Sharding: core c owns slab t in [48c, 48c+48).  The slab pos[t,:,:] serves
both c2p rows i=t and p2c columns j=t (same (384,768) slice, second index
means j resp. i).  p2c is produced column-sharded and moved to the row owners
with one small AllToAll (8 x 12x48x48), then re-transposed through the PE.
Everything else (projections, c2c, softmax, probs@v) is computed on the row
owner.  pos is cast to bf16 on the host so DMA-transpose (2-byte only) can
deliver it H-major (contraction dim on partitions) straight from HBM.
"""

import sys

sys.path.insert(0, "/opt/trn_rl_repo")

import math
import numpy as np
import ml_dtypes

import concourse.bass as bass
import concourse.bacc as bacc
import concourse.mybir as mybir
import concourse.tile as tile
from concourse.bass_utils import run_bass_kernel_spmd

BF16 = mybir.dt.bfloat16
F32 = mybir.dt.float32
AF = mybir.ActivationFunctionType

S = 384
H = 768
NH = 12
D = 64
NC = 8
TB = S // NC  # 48 rows per core
NCH = H // 128  # 6 chunks of the hidden dim


def build_module():
    nc = bacc.Bacc(trn_type="TRN2", num_devices=NC, debug=False)

    # ---- I/O ----
    pos_d = nc.dram_tensor("pos", [TB, 128, NCH, S], BF16, kind="ExternalInput")
    hsT_d = nc.dram_tensor("hsT", [H, S], BF16, kind="ExternalInput")
    hsTo_d = nc.dram_tensor("hsTo", [H, TB], BF16, kind="ExternalInput")
    wq_d = nc.dram_tensor("wq", [H, H], BF16, kind="ExternalInput")
    wk_d = nc.dram_tensor("wk", [H, H], BF16, kind="ExternalInput")
    wv_d = nc.dram_tensor("wv", [H, H], BF16, kind="ExternalInput")
    wpkT_d = nc.dram_tensor("wpkT", [H, H], BF16, kind="ExternalInput")
    wpqT_d = nc.dram_tensor("wpqT", [H, H], BF16, kind="ExternalInput")
    bqT_d = nc.dram_tensor("bqT", [128, NCH], F32, kind="ExternalInput")
    bkT_d = nc.dram_tensor("bkT", [128, NCH], F32, kind="ExternalInput")
    bv_d = nc.dram_tensor("bv", [H], F32, kind="ExternalInput")
    bpqd_d = nc.dram_tensor("bpqd", [128, NCH, NH], BF16, kind="ExternalInput")
    mask_d = nc.dram_tensor("maskrow", [S], F32, kind="ExternalInput")
    ident_d = nc.dram_tensor("ident", [128, 128], BF16, kind="ExternalInput")
    out_d = nc.dram_tensor("out", [TB, H], F32, kind="ExternalOutput")

    with tile.TileContext(nc) as tc:
        with (
            tc.tile_pool(name="const", bufs=1) as cpool,
            tc.tile_pool(name="work", bufs=1) as wpool,
            tc.tile_pool(name="posT", bufs=3) as ppool,
            tc.tile_pool(name="stg", bufs=4) as spool,
            tc.tile_pool(name="psum", bufs=8, space="PSUM") as pspool,
            tc.tile_pool(name="dram", bufs=1, space="DRAM") as dpool,
        ):
            # ---- load constants ----
            hsT = [cpool.tile([128, S], BF16, tag=f"hsT{m}", name=f"hsT{m}") for m in range(NCH)]
            hsTo = [cpool.tile([128, TB], BF16, tag=f"hsTo{m}", name=f"hsTo{m}") for m in range(NCH)]
            wq = [cpool.tile([128, H], BF16, tag=f"wq{m}", name=f"wq{m}") for m in range(NCH)]
            wk = [cpool.tile([128, H], BF16, tag=f"wk{m}", name=f"wk{m}") for m in range(NCH)]
            wv = [cpool.tile([128, H], BF16, tag=f"wv{m}", name=f"wv{m}") for m in range(NCH)]
            wpkT = [cpool.tile([128, H], BF16, tag=f"wpkT{m}", name=f"wpkT{m}") for m in range(NCH)]
            wpqT = [cpool.tile([128, H], BF16, tag=f"wpqT{m}", name=f"wpqT{m}") for m in range(NCH)]
            for m in range(NCH):
                r = slice(m * 128, (m + 1) * 128)
                nc.sync.dma_start(hsT[m][:], hsT_d[r, :])
                nc.sync.dma_start(hsTo[m][:], hsTo_d[r, :])
                nc.sync.dma_start(wq[m][:], wq_d[r, :])
                nc.sync.dma_start(wk[m][:], wk_d[r, :])
                nc.sync.dma_start(wv[m][:], wv_d[r, :])
                nc.sync.dma_start(wpkT[m][:], wpkT_d[r, :])
                nc.sync.dma_start(wpqT[m][:], wpqT_d[r, :])
            bqT = cpool.tile([128, NCH], F32, tag="bqT")
            bkT = cpool.tile([128, NCH], F32, tag="bkT")
            bpqd = cpool.tile([128, NCH, NH], BF16, tag="bpqd")
            ident = cpool.tile([128, 128], BF16, tag="ident")
            nc.sync.dma_start(bqT[:], bqT_d[:])
            nc.sync.dma_start(bkT[:], bkT_d[:])
            nc.sync.dma_start(bpqd[:], bpqd_d[:])
            nc.sync.dma_start(ident[:], ident_d[:])
            bvbc = cpool.tile([128, H], BF16, tag="bvbc")
            nc.gpsimd.dma_start(bvbc[:], bv_d[:].partition_broadcast(128))
            mask12 = cpool.tile([NH, S], F32, tag="mask12")
            nc.gpsimd.dma_start(mask12[:], mask_d[:].partition_broadcast(NH))

            # ---- projections ----
            # kT[hd, j] (full), qT_own[hd, i own], kT_own[hd, j own], v[j, hd]
            kT = [wpool.tile([128, S], BF16, tag=f"kT{m}", name=f"kT{m}") for m in range(NCH)]
            qTo = [wpool.tile([128, TB], BF16, tag=f"qTo{m}", name=f"qTo{m}") for m in range(NCH)]
            kTo = [wpool.tile([128, TB], BF16, tag=f"kTo{m}", name=f"kTo{m}") for m in range(NCH)]
            for m in range(NCH):
                ps = pspool.tile([128, S], F32, tag="ps")
                for c in range(NCH):
                    nc.tensor.matmul(
                        ps[:], wk[c][:, m * 128 : (m + 1) * 128], hsT[c][:],
                        start=(c == 0), stop=(c == NCH - 1),
                    )
                nc.vector.tensor_scalar_add(kT[m][:], ps[:], bkT[:, m : m + 1])
                pso = pspool.tile([128, TB], F32, tag="ps")
                for c in range(NCH):
                    nc.tensor.matmul(
                        pso[:], wq[c][:, m * 128 : (m + 1) * 128], hsTo[c][:],
                        start=(c == 0), stop=(c == NCH - 1),
                    )
                nc.vector.tensor_scalar_add(qTo[m][:], pso[:], bqT[:, m : m + 1])
                psk = pspool.tile([128, TB], F32, tag="ps")
                for c in range(NCH):
                    nc.tensor.matmul(
                        psk[:], wk[c][:, m * 128 : (m + 1) * 128], hsTo[c][:],
                        start=(c == 0), stop=(c == NCH - 1),
                    )
                nc.vector.tensor_scalar_add(kTo[m][:], psk[:], bkT[:, m : m + 1])

            v_sb = [wpool.tile([128, H], BF16, tag=f"v{jc}", name=f"v{jc}") for jc in range(3)]
            for jc in range(3):
                for nh in range(2):
                    ps = pspool.tile([128, S], F32, tag="ps")
                    for c in range(NCH):
                        nc.tensor.matmul(
                            ps[:],
                            hsT[c][:, jc * 128 : (jc + 1) * 128],
                            wv[c][:, nh * S : (nh + 1) * S],
                            start=(c == 0), stop=(c == NCH - 1),
                        )
                    nc.scalar.activation(v_sb[jc][:, nh * S : (nh + 1) * S], ps[:], AF.Copy)
                nc.vector.tensor_tensor(
                    v_sb[jc][:], v_sb[jc][:], bvbc[:], op=mybir.AluOpType.add
                )

            # ---- qkp[c_chunk][128, t, 24]: cols 0:12 qpk (Wpk.T q), 12:24 kpq ----
            qkp = [wpool.tile([128, TB, 2 * NH], BF16, tag=f"qkp{m}", name=f"qkp{m}") for m in range(NCH)]
            for h in range(NH):
                mh, oh = h // 2, (h % 2) * 64
                for m in range(NCH):
                    ps = pspool.tile([128, TB], F32, tag="ps")
                    nc.tensor.matmul(
                        ps[:],
                        wpkT[mh][oh : oh + 64, m * 128 : (m + 1) * 128],
                        qTo[mh][oh : oh + 64, :],
                    )
                    nc.scalar.activation(qkp[m][:, :, h], ps[:], AF.Copy)
                    ps2 = pspool.tile([128, TB], F32, tag="ps")
                    nc.tensor.matmul(
                        ps2[:],
                        wpqT[mh][oh : oh + 64, m * 128 : (m + 1) * 128],
                        kTo[mh][oh : oh + 64, :],
                    )
                    nc.scalar.activation(qkp[m][:, :, NH + h], ps2[:], AF.Copy)

            # ---- colbias[h, j] = (k[j] . bpq_h) / sqrt(D) + mask[j] ----
            pskb = pspool.tile([NH, S], F32, tag="ps")
            for m in range(NCH):
                nc.tensor.matmul(
                    pskb[:], bpqd[:, m, :], kT[m][:],
                    start=(m == 0), stop=(m == NCH - 1),
                )
            colbias = wpool.tile([NH, S], F32, tag="colbias")
            nc.scalar.activation(colbias[:], pskb[:], AF.Copy, scale=1.0 / math.sqrt(D))
            nc.vector.tensor_tensor(
                colbias[:], colbias[:], mask12[:], op=mybir.AluOpType.add
            )
            cb_dram = dpool.tile([NH, S], F32)
            nc.gpsimd.dma_start(cb_dram[:], colbias[:])
            colbias_bc = wpool.tile([TB, NH, S], F32, tag="colbias_bc")
            nc.gpsimd.dma_start(colbias_bc[:], cb_dram[:].partition_broadcast(TB))

            # ---- main loop over own slabs t: c2p rows + p2c cols ----
            c2p_dram = dpool.tile([TB, NH, S], BF16)
            HT = TB // 2
            a2a_in = [dpool.tile([NC, NH, HT, TB], BF16, name=f"a2ai{u}") for u in range(2)]
            a2a_out = [dpool.tile([NC, NH, HT, TB], BF16, name=f"a2ao{u}") for u in range(2)]
            for t in range(TB):
                posT = ppool.tile([128, NCH, S], BF16, tag="posT", name="posT")
                nc.sync.dma_start(posT[:], pos_d[t])
                ps = pspool.tile([2 * NH, S], F32, tag="ps")
                for m in range(NCH):
                    nc.tensor.matmul(
                        ps[:], qkp[m][:, t, :], posT[:, m, :],
                        start=(m == 0), stop=(m == NCH - 1),
                    )
                stage = spool.tile([2 * NH, S], BF16, tag="stage", name="stage")
                nc.scalar.activation(stage[:], ps[:], AF.Copy)
                nc.gpsimd.dma_start(c2p_dram[t], stage[0:NH, :])
                nc.gpsimd.dma_start(
                    a2a_in[t // HT][:, :, t % HT, :].rearrange("d h i -> h d i"),
                    stage[NH : 2 * NH, :].rearrange("h (d i) -> h d i", d=NC),
                )
                if t == HT - 1 or t == TB - 1:
                    nc.gpsimd.collective_compute(
                        "AllToAll",
                        mybir.AluOpType.bypass,
                        replica_groups=[list(range(NC))],
                        ins=[a2a_in[t // HT].opt()],
                        outs=[a2a_out[t // HT].opt()],
                    )

            # ---- c2c rows ----
            scores = wpool.tile([TB, NH, S], F32, tag="scores")
            for h in range(NH):
                mh, oh = h // 2, (h % 2) * 64
                ps = pspool.tile([TB, S], F32, tag="ps")
                nc.tensor.matmul(ps[:], qTo[mh][oh : oh + 64, :], kT[mh][oh : oh + 64, :])
                nc.scalar.activation(scores[:, h, :], ps[:], AF.Copy)

            # ---- p2c rows: transpose a2a_out [(s h t), il] -> [il, (s h t)] ----
            p2c_rows = [
                wpool.tile([TB, NC * NH * HT], BF16, tag=f"p2c_rows{u}", name=f"p2c_rows{u}")
                for u in range(2)
            ]
            NT = NC * NH * HT // 128  # 18 tiles of 128 rows per half
            for u in range(2):
                g2flat = a2a_out[u][:].rearrange("s h t i -> (s h t) i")
                for m in range(NT):
                    g2t = spool.tile([128, TB], BF16, tag="g2t", name="g2t")
                    nc.sync.dma_start(g2t[:], g2flat[m * 128 : (m + 1) * 128, :])
                    pst = pspool.tile([TB, 128], BF16, tag="ps")
                    nc.tensor.transpose(pst[:], g2t[:], ident[:])
                    nc.vector.tensor_copy(p2c_rows[u][:, m * 128 : (m + 1) * 128], pst[:])

            # ---- assemble scores rows, softmax ----
            c2p_rows = wpool.tile([TB, NH, S], BF16, tag="c2p_rows")
            nc.sync.dma_start(c2p_rows[:], c2p_dram[:])

            nc.vector.tensor_tensor(
                scores[:], scores[:], c2p_rows[:], op=mybir.AluOpType.add
            )
            # add each half: scores[i, h, s*TB + u*HT + t']
            for u in range(2):
                sc4 = scores[:].rearrange("i h (s t) -> i h s t", s=NC)[:, :, :, u * HT : (u + 1) * HT]
                nc.vector.tensor_tensor(
                    sc4,
                    sc4,
                    p2c_rows[u][:].rearrange("i (s h t) -> i h s t", s=NC, h=NH),
                    op=mybir.AluOpType.add,
                )
            # scores/sqrt(D) + colbias  (colbias = kb/sqrt(D) + mask)
            nc.scalar.activation(scores[:], scores[:], AF.Copy, scale=1.0 / math.sqrt(D))
            nc.vector.tensor_tensor(
                scores[:], scores[:], colbias_bc[:], op=mybir.AluOpType.add
            )

            negmax = wpool.tile([TB, NH], F32, tag="negmax")
            sums = wpool.tile([TB, NH], F32, tag="sums")
            recip = wpool.tile([TB, NH], F32, tag="recip")
            probs = wpool.tile([TB, NH, S], BF16, tag="probs")
            for h in range(NH):
                nc.vector.tensor_reduce(
                    out=negmax[:, h : h + 1], in_=scores[:, h, :],
                    op=mybir.AluOpType.max, axis=mybir.AxisListType.X, negate=True,
                )
                nc.scalar.activation(
                    probs[:, h, :], scores[:, h, :], AF.Exp,
                    bias=negmax[:, h : h + 1],
                    accum_out=sums[:, h : h + 1],
                )
            nc.vector.reciprocal(recip[:], sums[:])

            # ---- probs.T per head, then ctx = probs @ v ----
            ptile = [wpool.tile([128, NH, TB], BF16, tag=f"pt{jc}", name=f"pt{jc}") for jc in range(3)]
            for h in range(NH):
                for jc in range(3):
                    pst = pspool.tile([128, TB], BF16, tag="ps")
                    nc.tensor.transpose(
                        pst[:], probs[:, h, jc * 128 : (jc + 1) * 128], ident[0:TB, 0:TB]
                    )
                    nc.vector.tensor_copy(ptile[jc][:, h, :], pst[:])
            out_sb = wpool.tile([TB, H], F32, tag="out_sb")
            for h in range(NH):
                psc = pspool.tile([TB, D], F32, tag="ps")
                for jc in range(3):
                    nc.tensor.matmul(
                        psc[:], ptile[jc][:, h, :], v_sb[jc][:, h * D : (h + 1) * D],
                        start=(jc == 0), stop=(jc == 2),
                    )
                nc.vector.tensor_scalar_mul(
                    out_sb[:, h * D : (h + 1) * D], psc[:], recip[:, h : h + 1]
                )
            nc.sync.dma_start(out_d[:], out_sb[:])

    nc.compile()
    return nc


_NC_CACHE = None


def _prep_inputs(hidden_states, attention_mask, pos_emb, Wq, bq, Wk, bk, Wv, bv,
                 Wpk, bpk, Wpq, bpq):
    bf = ml_dtypes.bfloat16
    hs = np.ascontiguousarray(np.asarray(hidden_states, np.float32)[0])  # (S, H)
    hsT = np.ascontiguousarray(hs.T).astype(bf)
    wq = np.asarray(Wq, np.float32).astype(bf)
    wk = np.asarray(Wk, np.float32).astype(bf)
    wv = np.asarray(Wv, np.float32).astype(bf)
    wpkT = np.ascontiguousarray(np.asarray(Wpk, np.float32).T).astype(bf)
    wpqT = np.ascontiguousarray(np.asarray(Wpq, np.float32).T).astype(bf)
    bqT = np.ascontiguousarray(np.asarray(bq, np.float32).reshape(NCH, 128).T)
    bkT = np.ascontiguousarray(np.asarray(bk, np.float32).reshape(NCH, 128).T)
    bpq_f = np.asarray(bpq, np.float32)
    bpqd = np.zeros((128, NCH, NH), bf)
    for m in range(NCH):
        for half in range(2):
            h = 2 * m + half
            bpqd[64 * half : 64 * half + 64, m, h] = bpq_f[
                128 * m + 64 * half : 128 * m + 64 * half + 64
            ].astype(bf)
    mask_row = np.ascontiguousarray(np.asarray(attention_mask, np.float32)[0, 0, 0])
    ident = np.eye(128, dtype=bf)

    common = dict(
        hsT=hsT, wq=wq, wk=wk, wv=wv, wpkT=wpkT, wpqT=wpqT,
        bqT=bqT, bkT=bkT, bv=np.asarray(bv, np.float32),
        bpqd=bpqd, maskrow=mask_row, ident=ident,
    )
    in_maps = []
    pos0 = np.asarray(pos_emb)[0]  # (S, S, H) f32
    for c in range(NC):
        sl = slice(c * TB, (c + 1) * TB)
        m = dict(common)
        # [t, p, m, x] = pos[t0+t, x, 128m+p]: one DMA per t with contiguous
        # (NCH*S*2)B partition lines
        m["pos"] = (
            pos0[sl]
            .transpose(0, 2, 1)
            .reshape(TB, NCH, 128, S)
            .transpose(0, 2, 1, 3)
            .astype(bf)
        )
        m["hsTo"] = np.ascontiguousarray(hsT[:, sl])
        in_maps.append(m)
    return in_maps


def kernel(**inputs):
    global _NC_CACHE
    if _NC_CACHE is None:
        _NC_CACHE = build_module()
    nc = _NC_CACHE
    in_maps = _prep_inputs(**inputs)
    res = run_bass_kernel_spmd(nc, in_maps, core_ids=list(range(NC)))
    out = np.concatenate([r["out"] for r in res.results], axis=0)
    return out.reshape(1, S, H).astype(np.float32)


# revision 13
# speedup vs baseline: 1.4911x; 1.0948x over previous
"""Disentangled self-attention (DeBERTa-style) Trainium2 kernel, 8 NeuronCores.

Math restructuring: the reference projects pos_emb (S,S,H) through Wpk/Wpq
(~348 GFLOP).  Because each c2p/p2c score element only contracts the projected
vector with q/k, we instead contract q/k with the weight slices first:

    c2p[h,i,j] = sum_c qpk[h,i,c] * pos[i,j,c]   (+ q.bpk_h, const over j ->
                                                  cancels in softmax)
    p2c[h,i,j] = sum_c kpq[h,j,c] * pos[j,i,c]   + k[j].bpq_h
    qpk[h,i,c] = sum_d Wpk[c,hD+d] q[i,hD+d],  kpq likewise with Wpq/k

which drops the pos-side work to ~6 GFLOP and makes the single read of
pos_emb the bottleneck.

Sharding: core c owns slab t in [48c, 48c+48).  The slab pos[t,:,:] serves
both c2p rows i=t and p2c columns j=t (same (384,768) slice, second index
means j resp. i).  p2c is produced column-sharded and moved to the row owners
with one small AllToAll (8 x 12x48x48), then re-transposed through the PE.
Everything else (projections, c2c, softmax, probs@v) is computed on the row
owner.  pos is cast to bf16 on the host so DMA-transpose (2-byte only) can
deliver it H-major (contraction dim on partitions) straight from HBM.
"""

import sys

sys.path.insert(0, "/opt/trn_rl_repo")

import math
import numpy as np
import ml_dtypes

import concourse.bass as bass
import concourse.bacc as bacc
import concourse.mybir as mybir
import concourse.tile as tile
from concourse.bass_utils import run_bass_kernel_spmd

BF16 = mybir.dt.bfloat16
F32 = mybir.dt.float32
AF = mybir.ActivationFunctionType

S = 384
H = 768
NH = 12
D = 64
NC = 8
TB = S // NC  # 48 rows per core
NCH = H // 128  # 6 chunks of the hidden dim


def build_module():
    nc = bacc.Bacc(trn_type="TRN2", num_devices=NC, debug=False)

    # ---- I/O ----
    pos_d = nc.dram_tensor("pos", [TB, 128, NCH, S], BF16, kind="ExternalInput")
    hsT_d = nc.dram_tensor("hsT", [H, S], BF16, kind="ExternalInput")
    hsTo_d = nc.dram_tensor("hsTo", [H, TB], BF16, kind="ExternalInput")
    wq_d = nc.dram_tensor("wq", [H, H], BF16, kind="ExternalInput")
    wk_d = nc.dram_tensor("wk", [H, H], BF16, kind="ExternalInput")
    wv_d = nc.dram_tensor("wv", [H, H], BF16, kind="ExternalInput")
    wpkT_d = nc.dram_tensor("wpkT", [H, H], BF16, kind="ExternalInput")
    wpqT_d = nc.dram_tensor("wpqT", [H, H], BF16, kind="ExternalInput")
    bqT_d = nc.dram_tensor("bqT", [128, NCH], F32, kind="ExternalInput")
    bkT_d = nc.dram_tensor("bkT", [128, NCH], F32, kind="ExternalInput")
    bv_d = nc.dram_tensor("bv", [H], F32, kind="ExternalInput")
    bpqd_d = nc.dram_tensor("bpqd", [128, NCH, NH], BF16, kind="ExternalInput")
    mask_d = nc.dram_tensor("maskrow", [S], F32, kind="ExternalInput")
    ident_d = nc.dram_tensor("ident", [128, 128], BF16, kind="ExternalInput")
    out_d = nc.dram_tensor("out", [TB, H], F32, kind="ExternalOutput")

    with tile.TileContext(nc) as tc:
        with (
            tc.tile_pool(name="const", bufs=1) as cpool,
            tc.tile_pool(name="work", bufs=1) as wpool,
            tc.tile_pool(name="posT", bufs=4) as ppool,
            tc.tile_pool(name="stg", bufs=4) as spool,
            tc.tile_pool(name="psum", bufs=8, space="PSUM") as pspool,
            tc.tile_pool(name="dram", bufs=1, space="DRAM") as dpool,
        ):
            # ---- load constants ----
            hsT = [cpool.tile([128, S], BF16, tag=f"hsT{m}", name=f"hsT{m}") for m in range(NCH)]
            hsTo = [cpool.tile([128, TB], BF16, tag=f"hsTo{m}", name=f"hsTo{m}") for m in range(NCH)]
            wq = [cpool.tile([128, H], BF16, tag=f"wq{m}", name=f"wq{m}") for m in range(NCH)]
            wk = [cpool.tile([128, H], BF16, tag=f"wk{m}", name=f"wk{m}") for m in range(NCH)]
            wv = [cpool.tile([128, H], BF16, tag=f"wv{m}", name=f"wv{m}") for m in range(NCH)]
            wpkT = [cpool.tile([128, H], BF16, tag=f"wpkT{m}", name=f"wpkT{m}") for m in range(NCH)]
            wpqT = [cpool.tile([128, H], BF16, tag=f"wpqT{m}", name=f"wpqT{m}") for m in range(NCH)]
            for m in range(NCH):
                r = slice(m * 128, (m + 1) * 128)
                nc.sync.dma_start(hsT[m][:], hsT_d[r, :])
                nc.sync.dma_start(hsTo[m][:], hsTo_d[r, :])
                nc.sync.dma_start(wq[m][:], wq_d[r, :])
                nc.sync.dma_start(wk[m][:], wk_d[r, :])
                nc.sync.dma_start(wv[m][:], wv_d[r, :])
                nc.sync.dma_start(wpkT[m][:], wpkT_d[r, :])
                nc.sync.dma_start(wpqT[m][:], wpqT_d[r, :])
            bqT = cpool.tile([128, NCH], F32, tag="bqT")
            bkT = cpool.tile([128, NCH], F32, tag="bkT")
            bpqd = cpool.tile([128, NCH, NH], BF16, tag="bpqd")
            ident = cpool.tile([128, 128], BF16, tag="ident")
            nc.sync.dma_start(bqT[:], bqT_d[:])
            nc.sync.dma_start(bkT[:], bkT_d[:])
            nc.sync.dma_start(bpqd[:], bpqd_d[:])
            nc.sync.dma_start(ident[:], ident_d[:])
            bvbc = cpool.tile([128, H], BF16, tag="bvbc")
            nc.gpsimd.dma_start(bvbc[:], bv_d[:].partition_broadcast(128))
            mask12 = cpool.tile([NH, S], F32, tag="mask12")
            nc.gpsimd.dma_start(mask12[:], mask_d[:].partition_broadcast(NH))

            # ---- projections ----
            # kT[hd, j] (full), qT_own[hd, i own], kT_own[hd, j own], v[j, hd]
            kT = [wpool.tile([128, S], BF16, tag=f"kT{m}", name=f"kT{m}") for m in range(NCH)]
            qTo = [wpool.tile([128, TB], BF16, tag=f"qTo{m}", name=f"qTo{m}") for m in range(NCH)]
            kTo = [wpool.tile([128, TB], BF16, tag=f"kTo{m}", name=f"kTo{m}") for m in range(NCH)]
            for m in range(NCH):
                ps = pspool.tile([128, S], F32, tag="ps")
                for c in range(NCH):
                    nc.tensor.matmul(
                        ps[:], wk[c][:, m * 128 : (m + 1) * 128], hsT[c][:],
                        start=(c == 0), stop=(c == NCH - 1),
                    )
                nc.vector.tensor_scalar_add(kT[m][:], ps[:], bkT[:, m : m + 1])
                pso = pspool.tile([128, TB], F32, tag="ps")
                for c in range(NCH):
                    nc.tensor.matmul(
                        pso[:], wq[c][:, m * 128 : (m + 1) * 128], hsTo[c][:],
                        start=(c == 0), stop=(c == NCH - 1),
                    )
                nc.vector.tensor_scalar_add(qTo[m][:], pso[:], bqT[:, m : m + 1])
                psk = pspool.tile([128, TB], F32, tag="ps")
                for c in range(NCH):
                    nc.tensor.matmul(
                        psk[:], wk[c][:, m * 128 : (m + 1) * 128], hsTo[c][:],
                        start=(c == 0), stop=(c == NCH - 1),
                    )
                nc.vector.tensor_scalar_add(kTo[m][:], psk[:], bkT[:, m : m + 1])

            v_sb = [wpool.tile([128, H], BF16, tag=f"v{jc}", name=f"v{jc}") for jc in range(3)]
            for jc in range(3):
                for nh in range(2):
                    ps = pspool.tile([128, S], F32, tag="ps")
                    for c in range(NCH):
                        nc.tensor.matmul(
                            ps[:],
                            hsT[c][:, jc * 128 : (jc + 1) * 128],
                            wv[c][:, nh * S : (nh + 1) * S],
                            start=(c == 0), stop=(c == NCH - 1),
                        )
                    nc.scalar.activation(v_sb[jc][:, nh * S : (nh + 1) * S], ps[:], AF.Copy)
                nc.vector.tensor_tensor(
                    v_sb[jc][:], v_sb[jc][:], bvbc[:], op=mybir.AluOpType.add
                )

            # ---- qkp[c_chunk][128, t, 24]: cols 0:12 qpk (Wpk.T q), 12:24 kpq ----
            qkp = [wpool.tile([128, TB, 2 * NH], BF16, tag=f"qkp{m}", name=f"qkp{m}") for m in range(NCH)]
            for h in range(NH):
                mh, oh = h // 2, (h % 2) * 64
                for m in range(NCH):
                    ps = pspool.tile([128, TB], F32, tag="ps")
                    nc.tensor.matmul(
                        ps[:],
                        wpkT[mh][oh : oh + 64, m * 128 : (m + 1) * 128],
                        qTo[mh][oh : oh + 64, :],
                    )
                    nc.vector.tensor_copy(qkp[m][:, :, h], ps[:])
                    ps2 = pspool.tile([128, TB], F32, tag="ps")
                    nc.tensor.matmul(
                        ps2[:],
                        wpqT[mh][oh : oh + 64, m * 128 : (m + 1) * 128],
                        kTo[mh][oh : oh + 64, :],
                    )
                    nc.vector.tensor_copy(qkp[m][:, :, NH + h], ps2[:])

            # ---- colbias[h, j] = (k[j] . bpq_h) / sqrt(D) + mask[j] ----
            pskb = pspool.tile([NH, S], F32, tag="ps")
            for m in range(NCH):
                nc.tensor.matmul(
                    pskb[:], bpqd[:, m, :], kT[m][:],
                    start=(m == 0), stop=(m == NCH - 1),
                )
            colbias = wpool.tile([NH, S], F32, tag="colbias")
            nc.vector.tensor_tensor(colbias[:], pskb[:], mask12[:], op=mybir.AluOpType.add)
            cb_dram = dpool.tile([NH, S], F32)
            nc.gpsimd.dma_start(cb_dram[:], colbias[:])
            colbias_bc = wpool.tile([TB, NH, S], F32, tag="colbias_bc")
            nc.gpsimd.dma_start(colbias_bc[:], cb_dram[:].partition_broadcast(TB))

            # ---- main loop over own slabs t: c2p rows + p2c cols ----
            c2p_dram = dpool.tile([TB, NH, S], BF16)
            HT = TB // 2
            a2a_in = [dpool.tile([NC, NH, HT, TB], BF16, name=f"a2ai{u}") for u in range(2)]
            a2a_out = [dpool.tile([NC, NH, HT, TB], BF16, name=f"a2ao{u}") for u in range(2)]
            for t in range(TB):
                posT = ppool.tile([128, NCH, S], BF16, tag="posT", name="posT")
                nc.sync.dma_start(posT[:], pos_d[t])
                ps = pspool.tile([2 * NH, S], F32, tag="ps")
                for m in range(NCH):
                    nc.tensor.matmul(
                        ps[:], qkp[m][:, t, :], posT[:, m, :],
                        start=(m == 0), stop=(m == NCH - 1),
                    )
                stage = spool.tile([2 * NH, S], BF16, tag="stage", name="stage")
                nc.vector.tensor_copy(stage[:], ps[:])
                nc.gpsimd.dma_start(c2p_dram[t], stage[0:NH, :])
                nc.gpsimd.dma_start(
                    a2a_in[t // HT][:, :, t % HT, :].rearrange("d h i -> h d i"),
                    stage[NH : 2 * NH, :].rearrange("h (d i) -> h d i", d=NC),
                )
                if t == HT - 1 or t == TB - 1:
                    nc.gpsimd.collective_compute(
                        "AllToAll",
                        mybir.AluOpType.bypass,
                        replica_groups=[list(range(NC))],
                        ins=[a2a_in[t // HT].opt()],
                        outs=[a2a_out[t // HT].opt()],
                    )

            # ---- c2c rows ----
            scores = wpool.tile([TB, NH, S], F32, tag="scores")
            for h in range(NH):
                mh, oh = h // 2, (h % 2) * 64
                ps = pspool.tile([TB, S], F32, tag="ps")
                nc.tensor.matmul(ps[:], qTo[mh][oh : oh + 64, :], kT[mh][oh : oh + 64, :])
                nc.scalar.activation(scores[:, h, :], ps[:], AF.Copy)

            # ---- p2c rows: transpose a2a_out [(s h t), il] -> [il, (s h t)] ----
            p2c_rows = [
                wpool.tile([TB, NC * NH * HT], BF16, tag=f"p2c_rows{u}", name=f"p2c_rows{u}")
                for u in range(2)
            ]
            NT = NC * NH * HT // 128  # 18 tiles of 128 rows per half
            for u in range(2):
                g2flat = a2a_out[u][:].rearrange("s h t i -> (s h t) i")
                for m in range(NT):
                    g2t = spool.tile([128, TB], BF16, tag="g2t", name="g2t")
                    nc.sync.dma_start(g2t[:], g2flat[m * 128 : (m + 1) * 128, :])
                    pst = pspool.tile([TB, 128], BF16, tag="ps")
                    nc.tensor.transpose(pst[:], g2t[:], ident[:])
                    nc.vector.tensor_copy(p2c_rows[u][:, m * 128 : (m + 1) * 128], pst[:])

            # ---- assemble scores rows, softmax ----
            c2p_rows = wpool.tile([TB, NH, S], BF16, tag="c2p_rows")
            nc.sync.dma_start(c2p_rows[:], c2p_dram[:])

            nc.vector.tensor_tensor(
                scores[:], scores[:], c2p_rows[:], op=mybir.AluOpType.add
            )
            # add each half: scores[i, h, s*TB + u*HT + t']
            for u in range(2):
                sc4 = scores[:].rearrange("i h (s t) -> i h s t", s=NC)[:, :, :, u * HT : (u + 1) * HT]
                nc.vector.tensor_tensor(
                    sc4,
                    sc4,
                    p2c_rows[u][:].rearrange("i (s h t) -> i h s t", s=NC, h=NH),
                    op=mybir.AluOpType.add,
                )
            # scores += colbias (kb + 8*mask); the 1/sqrt(D) folds into exp scale
            nc.vector.tensor_tensor(
                scores[:], scores[:], colbias_bc[:], op=mybir.AluOpType.add
            )

            negmax = wpool.tile([TB, NH], F32, tag="negmax")
            sums = wpool.tile([TB, NH], F32, tag="sums")
            recip = wpool.tile([TB, NH], F32, tag="recip")
            probs = wpool.tile([TB, NH, S], BF16, tag="probs")
            for h in range(NH):
                nc.vector.tensor_reduce(
                    out=negmax[:, h : h + 1], in_=scores[:, h, :],
                    op=mybir.AluOpType.max, axis=mybir.AxisListType.X, negate=True,
                )
            nc.vector.tensor_scalar_mul(negmax[:], negmax[:], 1.0 / math.sqrt(D))
            for h in range(NH):
                nc.scalar.activation(
                    probs[:, h, :], scores[:, h, :], AF.Exp,
                    bias=negmax[:, h : h + 1],
                    scale=1.0 / math.sqrt(D),
                    accum_out=sums[:, h : h + 1],
                )
            nc.vector.reciprocal(recip[:], sums[:])

            # ---- probs.T per head, then ctx = probs @ v ----
            ptile = [wpool.tile([128, NH, TB], BF16, tag=f"pt{jc}", name=f"pt{jc}") for jc in range(3)]
            for h in range(NH):
                for jc in range(3):
                    pst = pspool.tile([128, TB], BF16, tag="ps")
                    nc.tensor.transpose(
                        pst[:], probs[:, h, jc * 128 : (jc + 1) * 128], ident[0:TB, 0:TB]
                    )
                    nc.vector.tensor_copy(ptile[jc][:, h, :], pst[:])
            out_sb = wpool.tile([TB, H], F32, tag="out_sb")
            for h in range(NH):
                psc = pspool.tile([TB, D], F32, tag="ps")
                for jc in range(3):
                    nc.tensor.matmul(
                        psc[:], ptile[jc][:, h, :], v_sb[jc][:, h * D : (h + 1) * D],
                        start=(jc == 0), stop=(jc == 2),
                    )
                nc.vector.tensor_scalar_mul(
                    out_sb[:, h * D : (h + 1) * D], psc[:], recip[:, h : h + 1]
                )
            nc.sync.dma_start(out_d[:], out_sb[:])

    nc.compile()
    return nc


_NC_CACHE = None


def _prep_inputs(hidden_states, attention_mask, pos_emb, Wq, bq, Wk, bk, Wv, bv,
                 Wpk, bpk, Wpq, bpq):
    bf = ml_dtypes.bfloat16
    hs = np.ascontiguousarray(np.asarray(hidden_states, np.float32)[0])  # (S, H)
    hsT = np.ascontiguousarray(hs.T).astype(bf)
    wq = np.asarray(Wq, np.float32).astype(bf)
    wk = np.asarray(Wk, np.float32).astype(bf)
    wv = np.asarray(Wv, np.float32).astype(bf)
    wpkT = np.ascontiguousarray(np.asarray(Wpk, np.float32).T).astype(bf)
    wpqT = np.ascontiguousarray(np.asarray(Wpq, np.float32).T).astype(bf)
    bqT = np.ascontiguousarray(np.asarray(bq, np.float32).reshape(NCH, 128).T)
    bkT = np.ascontiguousarray(np.asarray(bk, np.float32).reshape(NCH, 128).T)
    bpq_f = np.asarray(bpq, np.float32)
    bpqd = np.zeros((128, NCH, NH), bf)
    for m in range(NCH):
        for half in range(2):
            h = 2 * m + half
            bpqd[64 * half : 64 * half + 64, m, h] = bpq_f[
                128 * m + 64 * half : 128 * m + 64 * half + 64
            ].astype(bf)
    mask_row = np.ascontiguousarray(np.asarray(attention_mask, np.float32)[0, 0, 0]) * math.sqrt(D)
    ident = np.eye(128, dtype=bf)

    common = dict(
        hsT=hsT, wq=wq, wk=wk, wv=wv, wpkT=wpkT, wpqT=wpqT,
        bqT=bqT, bkT=bkT, bv=np.asarray(bv, np.float32),
        bpqd=bpqd, maskrow=mask_row, ident=ident,
    )
    in_maps = []
    pos0 = np.asarray(pos_emb)[0]  # (S, S, H) f32
    for c in range(NC):
        sl = slice(c * TB, (c + 1) * TB)
        m = dict(common)
        # [t, p, m, x] = pos[t0+t, x, 128m+p]: one DMA per t with contiguous
        # (NCH*S*2)B partition lines
        m["pos"] = (
            pos0[sl]
            .transpose(0, 2, 1)
            .reshape(TB, NCH, 128, S)
            .transpose(0, 2, 1, 3)
            .astype(bf)
        )
        m["hsTo"] = np.ascontiguousarray(hsT[:, sl])
        in_maps.append(m)
    return in_maps


def kernel(**inputs):
    global _NC_CACHE
    if _NC_CACHE is None:
        _NC_CACHE = build_module()
    nc = _NC_CACHE
    in_maps = _prep_inputs(**inputs)
    res = run_bass_kernel_spmd(nc, in_maps, core_ids=list(range(NC)))
    out = np.concatenate([r["out"] for r in res.results], axis=0)
    return out.reshape(1, S, H).astype(np.float32)


# revision 16
# speedup vs baseline: 1.6019x; 1.0743x over previous
"""Disentangled self-attention (DeBERTa-style) Trainium2 kernel, 8 NeuronCores.

Math restructuring: the reference projects pos_emb (S,S,H) through Wpk/Wpq
(~348 GFLOP).  Because each c2p/p2c score element only contracts the projected
vector with q/k, we instead contract q/k with the weight slices first:

    c2p[h,i,j] = sum_c qpk[h,i,c] * pos[i,j,c]   (+ q.bpk_h, const over j ->
                                                  cancels in softmax)
    p2c[h,i,j] = sum_c kpq[h,j,c] * pos[j,i,c]   + k[j].bpq_h
    qpk[h,i,c] = sum_d Wpk[c,hD+d] q[i,hD+d],  kpq likewise with Wpq/k

which drops the pos-side work to ~6 GFLOP and makes the single read of
pos_emb the bottleneck.

Sharding: core c owns slab t in [48c, 48c+48).  The slab pos[t,:,:] serves
both c2p rows i=t and p2c columns j=t (same (384,768) slice, second index
means j resp. i).  p2c is produced column-sharded and moved to the row owners
with two small AllToAlls (first half overlaps the second half of the main
loop), then re-transposed through the PE.  Everything else (projections, c2c,
softmax, probs@v) is computed on the row owner; the full-k/v/c2c work is
placed after the main loop so it runs in the shadow of the second AllToAll.
pos is cast to bf16 and pre-transposed/interleaved on the host so each slab
is one contiguous-per-partition DMA.
"""

import sys

sys.path.insert(0, "/opt/trn_rl_repo")

import math
import numpy as np
import ml_dtypes

import concourse.bass as bass
import concourse.bacc as bacc
import concourse.mybir as mybir
import concourse.tile as tile
from concourse.bass_utils import run_bass_kernel_spmd

BF16 = mybir.dt.bfloat16
F32 = mybir.dt.float32
AF = mybir.ActivationFunctionType
ADD = mybir.AluOpType.add

S = 384
H = 768
NH = 12
D = 64
NC = 8
TB = S // NC  # 48 rows per core
NCH = H // 128  # 6 chunks of the hidden dim
HT = TB // 2  # AllToAll half
TPD = 2  # t-slabs per pos DMA


def build_module():
    nc = bacc.Bacc(trn_type="TRN2", num_devices=NC, debug=False)

    # ---- I/O ----
    pos_d = nc.dram_tensor("pos", [TB, 128, NCH, S], BF16, kind="ExternalInput")
    hsT_d = nc.dram_tensor("hsT", [128, NCH, S], BF16, kind="ExternalInput")
    hsTo_d = nc.dram_tensor("hsTo", [128, NCH, TB], BF16, kind="ExternalInput")
    wq_d = nc.dram_tensor("wq", [128, NCH, H], BF16, kind="ExternalInput")
    wk_d = nc.dram_tensor("wk", [128, NCH, H], BF16, kind="ExternalInput")
    wv_d = nc.dram_tensor("wv", [128, NCH, H], BF16, kind="ExternalInput")
    wpkT_d = nc.dram_tensor("wpkT", [128, NCH, H], BF16, kind="ExternalInput")
    wpqT_d = nc.dram_tensor("wpqT", [128, NCH, H], BF16, kind="ExternalInput")
    bqT_d = nc.dram_tensor("bqT", [128, NCH], F32, kind="ExternalInput")
    bkT_d = nc.dram_tensor("bkT", [128, NCH], F32, kind="ExternalInput")
    bv_d = nc.dram_tensor("bv", [H], F32, kind="ExternalInput")
    bpqd_d = nc.dram_tensor("bpqd", [128, NCH, NH], BF16, kind="ExternalInput")
    mask_d = nc.dram_tensor("maskrow", [S], F32, kind="ExternalInput")
    ident_d = nc.dram_tensor("ident", [128, 128], BF16, kind="ExternalInput")
    out_d = nc.dram_tensor("out", [TB, H], F32, kind="ExternalOutput")

    with tile.TileContext(nc) as tc:
        with (
            tc.tile_pool(name="const", bufs=1) as cpool,
            tc.tile_pool(name="work", bufs=1) as wpool,
            tc.tile_pool(name="posT", bufs=3) as ppool,
            tc.tile_pool(name="stg", bufs=4) as spool,
            tc.tile_pool(name="psum", bufs=8, space="PSUM") as pspool,
            tc.tile_pool(name="dram", bufs=1, space="DRAM") as dpool,
        ):
            # ---- early constants (needed for qkp before the main loop) ----
            hsTo = cpool.tile([128, NCH, TB], BF16, tag="hsTo")
            wq = cpool.tile([128, NCH, H], BF16, tag="wq")
            wk = cpool.tile([128, NCH, H], BF16, tag="wk")
            wpkT = cpool.tile([128, NCH, H], BF16, tag="wpkT")
            wpqT = cpool.tile([128, NCH, H], BF16, tag="wpqT")
            bqT = cpool.tile([128, NCH], F32, tag="bqT")
            bkT = cpool.tile([128, NCH], F32, tag="bkT")
            bpqd = cpool.tile([128, NCH, NH], BF16, tag="bpqd")
            ident = cpool.tile([128, 128], BF16, tag="ident")
            nc.sync.dma_start(hsTo[:], hsTo_d[:])
            nc.sync.dma_start(wq[:], wq_d[:])
            nc.sync.dma_start(wk[:], wk_d[:])
            nc.sync.dma_start(wpkT[:], wpkT_d[:])
            nc.sync.dma_start(wpqT[:], wpqT_d[:])
            nc.sync.dma_start(bqT[:], bqT_d[:])
            nc.sync.dma_start(bkT[:], bkT_d[:])
            nc.sync.dma_start(bpqd[:], bpqd_d[:])
            nc.sync.dma_start(ident[:], ident_d[:])
            bvbc = cpool.tile([128, H], BF16, tag="bvbc")
            nc.gpsimd.dma_start(bvbc[:], bv_d[:].partition_broadcast(128))
            mask12 = cpool.tile([NH, S], F32, tag="mask12")
            nc.gpsimd.dma_start(mask12[:], mask_d[:].partition_broadcast(NH))

            # ---- own-row projections qT_own / kT_own ----
            qTo = wpool.tile([128, NCH, TB], BF16, tag="qTo")
            kTo = wpool.tile([128, NCH, TB], BF16, tag="kTo")
            for m in range(NCH):
                pso = pspool.tile([128, TB], F32, tag="ps")
                for c in range(NCH):
                    nc.tensor.matmul(
                        pso[:], wq[:, c, m * 128 : (m + 1) * 128], hsTo[:, c, :],
                        start=(c == 0), stop=(c == NCH - 1),
                    )
                nc.vector.tensor_scalar_add(qTo[:, m, :], pso[:], bqT[:, m : m + 1])
                psk = pspool.tile([128, TB], F32, tag="ps")
                for c in range(NCH):
                    nc.tensor.matmul(
                        psk[:], wk[:, c, m * 128 : (m + 1) * 128], hsTo[:, c, :],
                        start=(c == 0), stop=(c == NCH - 1),
                    )
                nc.vector.tensor_scalar_add(kTo[:, m, :], psk[:], bkT[:, m : m + 1])

            # ---- qkp[c_chunk][128, t, 24]: cols 0:12 qpk (Wpk.T q), 12:24 kpq --
            # grouped 4 heads per PSUM tile -> one cast per group
            qkp = [
                wpool.tile([128, TB, 2 * NH], BF16, tag=f"qkp{m}", name=f"qkp{m}")
                for m in range(NCH)
            ]
            for m in range(NCH):
                for h in range(NH):
                    mh, oh = h // 2, (h % 2) * 64
                    ps1 = pspool.tile([128, TB], F32, tag="ps")
                    nc.tensor.matmul(
                        ps1[:],
                        wpkT[oh : oh + 64, mh, m * 128 : (m + 1) * 128],
                        qTo[oh : oh + 64, mh, :],
                    )
                    nc.scalar.activation(qkp[m][:, :, h], ps1[:], AF.Copy)
                    ps2 = pspool.tile([128, TB], F32, tag="ps")
                    nc.tensor.matmul(
                        ps2[:],
                        wpqT[oh : oh + 64, mh, m * 128 : (m + 1) * 128],
                        kTo[oh : oh + 64, mh, :],
                    )
                    nc.vector.tensor_copy(qkp[m][:, :, NH + h], ps2[:])

            # ---- main loop over own slabs t: c2p rows + p2c cols ----
            c2p_dram = dpool.tile([TB, NH, S], BF16)
            a2a_in = [dpool.tile([NC, NH, HT, TB], BF16, name=f"a2ai{u}") for u in range(2)]
            a2a_out = [dpool.tile([NC, NH, HT, TB], BF16, name=f"a2ao{u}") for u in range(2)]
            for t in range(TB):
                    posT = ppool.tile([128, NCH, S], BF16, tag="posT", name="posT")
                    nc.sync.dma_start(posT[:], pos_d[t])
                    ps = pspool.tile([2 * NH, S], F32, tag="ps")
                    for m in range(NCH):
                        nc.tensor.matmul(
                            ps[:], qkp[m][:, t, :], posT[:, m, :],
                            start=(m == 0), stop=(m == NCH - 1),
                        )
                    stage = spool.tile([2 * NH, S], BF16, tag="stage", name="stage")
                    nc.vector.tensor_copy(stage[:], ps[:])
                    nc.gpsimd.dma_start(c2p_dram[t], stage[0:NH, :])
                    nc.gpsimd.dma_start(
                        a2a_in[t // HT][:, :, t % HT, :].rearrange("d h i -> h d i"),
                        stage[NH : 2 * NH, :].rearrange("h (d i) -> h d i", d=NC),
                    )
                    if t == HT - 1 or t == TB - 1:
                        nc.gpsimd.collective_compute(
                            "AllToAll",
                            mybir.AluOpType.bypass,
                            replica_groups=[list(range(NC))],
                            ins=[a2a_in[t // HT].opt()],
                            outs=[a2a_out[t // HT].opt()],
                        )

            # ---- late constants (only needed after the loop) ----
            hsT = cpool.tile([128, NCH, S], BF16, tag="hsT")
            wv = cpool.tile([128, NCH, H], BF16, tag="wv")
            nc.sync.dma_start(hsT[:], hsT_d[:])
            nc.sync.dma_start(wv[:], wv_d[:])

            # ---- p2c rows half 0: transpose a2a_out [(s h t), il] -> [il, ...]
            p2c_rows = [
                wpool.tile([TB, NC * NH * HT], BF16, tag=f"p2c_rows{u}", name=f"p2c_rows{u}")
                for u in range(2)
            ]
            NT = NC * NH * HT // 128  # 18 tiles of 128 rows per half
            g2 = [
                wpool.tile([128, NT, TB], BF16, tag=f"g2_{u}", name=f"g2_{u}")
                for u in range(2)
            ]

            def transpose_half(u):
                nc.sync.dma_start(
                    g2[u][:],
                    a2a_out[u][:]
                    .rearrange("s h t i -> (s h t) i")
                    .rearrange("(m p) i -> p m i", p=128),
                )
                for m in range(NT):
                    pst = pspool.tile([TB, 128], BF16, tag="ps")
                    nc.tensor.transpose(pst[:], g2[u][:, m, :], ident[:])
                    nc.vector.tensor_copy(p2c_rows[u][:, m * 128 : (m + 1) * 128], pst[:])

            transpose_half(0)

            # ---- full kT, v, c2c, kb (overlaps the second AllToAll) ----
            kT = wpool.tile([128, NCH, S], BF16, tag="kT")
            for m in range(NCH):
                ps = pspool.tile([128, S], F32, tag="ps")
                for c in range(NCH):
                    nc.tensor.matmul(
                        ps[:], wk[:, c, m * 128 : (m + 1) * 128], hsT[:, c, :],
                        start=(c == 0), stop=(c == NCH - 1),
                    )
                nc.vector.tensor_scalar_add(kT[:, m, :], ps[:], bkT[:, m : m + 1])

            v_sb = wpool.tile([128, 3, H], BF16, tag="v_sb")
            for jc in range(3):
                for nh in range(2):
                    ps = pspool.tile([128, S], F32, tag="ps")
                    for c in range(NCH):
                        nc.tensor.matmul(
                            ps[:],
                            hsT[:, c, jc * 128 : (jc + 1) * 128],
                            wv[:, c, nh * S : (nh + 1) * S],
                            start=(c == 0), stop=(c == NCH - 1),
                        )
                    nc.scalar.activation(v_sb[:, jc, nh * S : (nh + 1) * S], ps[:], AF.Copy)
                nc.vector.tensor_tensor(v_sb[:, jc, :], v_sb[:, jc, :], bvbc[:], op=ADD)

            # colbias[h, j] = k[j].bpq_h + sqrt(D)*mask[j]; 1/sqrt(D) folds into exp
            pskb = pspool.tile([NH, S], F32, tag="ps")
            for m in range(NCH):
                nc.tensor.matmul(
                    pskb[:], bpqd[:, m, :], kT[:, m, :],
                    start=(m == 0), stop=(m == NCH - 1),
                )
            colbias = wpool.tile([NH, S], F32, tag="colbias")
            nc.vector.tensor_tensor(colbias[:], pskb[:], mask12[:], op=ADD)
            cb_dram = dpool.tile([NH, S], F32)
            nc.gpsimd.dma_start(cb_dram[:], colbias[:])
            colbias_bc = wpool.tile([TB, NH, S], F32, tag="colbias_bc")
            nc.gpsimd.dma_start(colbias_bc[:], cb_dram[:].partition_broadcast(TB))

            # ---- c2c rows (+ colbias in the same pass) ----
            scores = wpool.tile([TB, NH, S], F32, tag="scores")
            for h in range(NH):
                mh, oh = h // 2, (h % 2) * 64
                ps = pspool.tile([TB, S], F32, tag="ps")
                nc.tensor.matmul(ps[:], qTo[oh : oh + 64, mh, :], kT[oh : oh + 64, mh, :])
                nc.vector.tensor_tensor(scores[:, h, :], ps[:], colbias_bc[:, h, :], op=ADD)

            transpose_half(1)

            # ---- assemble scores rows, softmax ----
            c2p_rows = wpool.tile([TB, NH, S], BF16, tag="c2p_rows")
            nc.sync.dma_start(c2p_rows[:], c2p_dram[:])
            nc.vector.tensor_tensor(scores[:], scores[:], c2p_rows[:], op=ADD)
            for u in range(2):
                sc4 = scores[:].rearrange("i h (s t) -> i h s t", s=NC)[
                    :, :, :, u * HT : (u + 1) * HT
                ]
                nc.vector.tensor_tensor(
                    sc4,
                    sc4,
                    p2c_rows[u][:].rearrange("i (s h t) -> i h s t", s=NC, h=NH),
                    op=ADD,
                )

            negmax = wpool.tile([TB, NH], F32, tag="negmax")
            sums = wpool.tile([TB, NH], F32, tag="sums")
            recip = wpool.tile([TB, NH], F32, tag="recip")
            probs = wpool.tile([TB, NH, S], BF16, tag="probs")
            for h in range(NH):
                nc.vector.tensor_reduce(
                    out=negmax[:, h : h + 1], in_=scores[:, h, :],
                    op=mybir.AluOpType.max, axis=mybir.AxisListType.X, negate=True,
                )
            nc.vector.tensor_scalar_mul(negmax[:], negmax[:], 1.0 / math.sqrt(D))
            for h in range(NH):
                nc.scalar.activation(
                    probs[:, h, :], scores[:, h, :], AF.Exp,
                    bias=negmax[:, h : h + 1],
                    scale=1.0 / math.sqrt(D),
                    accum_out=sums[:, h : h + 1],
                )
            nc.vector.reciprocal(recip[:], sums[:])

            # ---- probs.T per head, then ctx = probs @ v ----
            ptile = wpool.tile([128, 3, NH, TB], BF16, tag="ptile")
            for h in range(NH):
                for jc in range(3):
                    pst = pspool.tile([128, TB], BF16, tag="ps")
                    nc.tensor.transpose(
                        pst[:], probs[:, h, jc * 128 : (jc + 1) * 128], ident[0:TB, 0:TB]
                    )
                    nc.vector.tensor_copy(ptile[:, jc, h, :], pst[:])
            out_sb = wpool.tile([TB, H], F32, tag="out_sb")
            for h in range(NH):
                psc = pspool.tile([TB, D], F32, tag="ps")
                for jc in range(3):
                    nc.tensor.matmul(
                        psc[:], ptile[:, jc, h, :], v_sb[:, jc, h * D : (h + 1) * D],
                        start=(jc == 0), stop=(jc == 2),
                    )
                nc.vector.tensor_scalar_mul(
                    out_sb[:, h * D : (h + 1) * D], psc[:], recip[:, h : h + 1]
                )
            nc.sync.dma_start(out_d[:], out_sb[:])

    nc.compile()
    return nc


_NC_CACHE = None


def _chunked(w):
    """[H, X] f32 -> [128, NCH, X] bf16 with [p, m, x] = w[128m+p, x]."""
    bf = ml_dtypes.bfloat16
    X = w.shape[1]
    return np.ascontiguousarray(
        np.asarray(w, np.float32).reshape(NCH, 128, X).transpose(1, 0, 2)
    ).astype(bf)


def _prep_inputs(hidden_states, attention_mask, pos_emb, Wq, bq, Wk, bk, Wv, bv,
                 Wpk, bpk, Wpq, bpq):
    bf = ml_dtypes.bfloat16
    hs = np.ascontiguousarray(np.asarray(hidden_states, np.float32)[0])  # (S, H)
    hsT = np.ascontiguousarray(hs.T)  # (H, S) f32
    bqT = np.ascontiguousarray(np.asarray(bq, np.float32).reshape(NCH, 128).T)
    bkT = np.ascontiguousarray(np.asarray(bk, np.float32).reshape(NCH, 128).T)
    bpq_f = np.asarray(bpq, np.float32)
    bpqd = np.zeros((128, NCH, NH), bf)
    for m in range(NCH):
        for half in range(2):
            h = 2 * m + half
            bpqd[64 * half : 64 * half + 64, m, h] = bpq_f[
                128 * m + 64 * half : 128 * m + 64 * half + 64
            ].astype(bf)
    mask_row = (
        np.ascontiguousarray(np.asarray(attention_mask, np.float32)[0, 0, 0])
        * math.sqrt(D)
    )
    ident = np.eye(128, dtype=bf)

    common = dict(
        hsT=_chunked(hsT),
        wq=_chunked(np.asarray(Wq)), wk=_chunked(np.asarray(Wk)),
        wv=_chunked(np.asarray(Wv)),
        wpkT=_chunked(np.ascontiguousarray(np.asarray(Wpk, np.float32).T)),
        wpqT=_chunked(np.ascontiguousarray(np.asarray(Wpq, np.float32).T)),
        bqT=bqT, bkT=bkT, bv=np.asarray(bv, np.float32),
        bpqd=bpqd, maskrow=mask_row, ident=ident,
    )
    in_maps = []
    pos0 = np.asarray(pos_emb)[0]  # (S, S, H) f32
    for c in range(NC):
        sl = slice(c * TB, (c + 1) * TB)
        m = dict(common)
        # [t, p, mm, x] = pos[t0+t, x, 128*mm+p]: one DMA per slab pair with
        # contiguous (NCH*S*2)B partition lines
        m["pos"] = (
            pos0[sl]
            .transpose(0, 2, 1)
            .reshape(TB, NCH, 128, S)
            .transpose(0, 2, 1, 3)
            .astype(bf)
        )
        m["hsTo"] = _chunked(hsT[:, sl])
        in_maps.append(m)
    return in_maps


def kernel(**inputs):
    global _NC_CACHE
    if _NC_CACHE is None:
        _NC_CACHE = build_module()
    nc = _NC_CACHE
    in_maps = _prep_inputs(**inputs)
    res = run_bass_kernel_spmd(nc, in_maps, core_ids=list(range(NC)))
    out = np.concatenate([r["out"] for r in res.results], axis=0)
    return out.reshape(1, S, H).astype(np.float32)


# revision 18
# speedup vs baseline: 1.7034x; 1.0634x over previous
"""Disentangled self-attention (DeBERTa-style) Trainium2 kernel, 8 NeuronCores.

Math restructuring: the reference projects pos_emb (S,S,H) through Wpk/Wpq
(~348 GFLOP).  Because each c2p/p2c score element only contracts the projected
vector with q/k, we instead contract q/k with the weight slices first:

    c2p[h,i,j] = sum_c qpk[h,i,c] * pos[i,j,c]   (+ q.bpk_h, const over j ->
                                                  cancels in softmax)
    p2c[h,i,j] = sum_c kpq[h,j,c] * pos[j,i,c]   + k[j].bpq_h
    qpk[h,i,c] = sum_d Wpk[c,hD+d] q[i,hD+d],  kpq likewise with Wpq/k

which drops the pos-side work to ~6 GFLOP and makes the single read of
pos_emb the bottleneck.

Sharding: core c owns slab t in [48c, 48c+48).  The slab pos[t,:,:] serves
both c2p rows i=t and p2c columns j=t (same (384,768) slice, second index
means j resp. i).  p2c is produced column-sharded and moved to the row owners
with two small AllToAlls (first half overlaps the second half of the main
loop), then re-transposed through the PE.  Everything else (projections, c2c,
softmax, probs@v) is computed on the row owner; the full-k/v/c2c work is
placed after the main loop so it runs in the shadow of the second AllToAll.
pos is cast to bf16 and pre-transposed/interleaved on the host so each slab
is one contiguous-per-partition DMA.
"""

import sys

sys.path.insert(0, "/opt/trn_rl_repo")

import math
import numpy as np
import ml_dtypes

import concourse.bass as bass
import concourse.bacc as bacc
import concourse.mybir as mybir
import concourse.tile as tile
from concourse.bass_utils import run_bass_kernel_spmd

BF16 = mybir.dt.bfloat16
F32 = mybir.dt.float32
AF = mybir.ActivationFunctionType
ADD = mybir.AluOpType.add

S = 384
H = 768
NH = 12
D = 64
NC = 8
TB = S // NC  # 48 rows per core
NCH = H // 128  # 6 chunks of the hidden dim
HT = TB // 2  # AllToAll half
TPD = 2  # t-slabs per pos DMA


def build_module():
    nc = bacc.Bacc(trn_type="TRN2", num_devices=NC, debug=False)

    # ---- I/O ----
    pos_d = nc.dram_tensor("pos", [TB // 2, 128, 2, NCH, S], BF16, kind="ExternalInput")
    hsT_d = nc.dram_tensor("hsT", [128, NCH, S], BF16, kind="ExternalInput")
    hsTo_d = nc.dram_tensor("hsTo", [128, NCH, TB], BF16, kind="ExternalInput")
    wq_d = nc.dram_tensor("wq", [128, NCH, H], BF16, kind="ExternalInput")
    wk_d = nc.dram_tensor("wk", [128, NCH, H], BF16, kind="ExternalInput")
    wv_d = nc.dram_tensor("wv", [128, NCH, H], BF16, kind="ExternalInput")
    wpkT_d = nc.dram_tensor("wpkT", [128, NCH, H], BF16, kind="ExternalInput")
    wpqT_d = nc.dram_tensor("wpqT", [128, NCH, H], BF16, kind="ExternalInput")
    bqT_d = nc.dram_tensor("bqT", [128, NCH], F32, kind="ExternalInput")
    bkT_d = nc.dram_tensor("bkT", [128, NCH], F32, kind="ExternalInput")
    bv_d = nc.dram_tensor("bv", [H], F32, kind="ExternalInput")
    bpqd_d = nc.dram_tensor("bpqd", [128, NCH, NH], BF16, kind="ExternalInput")
    mask_d = nc.dram_tensor("maskrow", [S], F32, kind="ExternalInput")
    ident_d = nc.dram_tensor("ident", [128, 128], BF16, kind="ExternalInput")
    out_d = nc.dram_tensor("out", [TB, H], F32, kind="ExternalOutput")

    with tile.TileContext(nc) as tc:
        with (
            tc.tile_pool(name="const", bufs=1) as cpool,
            tc.tile_pool(name="work", bufs=1) as wpool,
            tc.tile_pool(name="posT", bufs=3) as ppool,
            tc.tile_pool(name="stg", bufs=4) as spool,
            tc.tile_pool(name="psum", bufs=8, space="PSUM") as pspool,
            tc.tile_pool(name="dram", bufs=1, space="DRAM") as dpool,
        ):
            # ---- early constants (needed for qkp before the main loop) ----
            hsTo = cpool.tile([128, NCH, TB], BF16, tag="hsTo")
            wq = cpool.tile([128, NCH, H], BF16, tag="wq")
            wk = cpool.tile([128, NCH, H], BF16, tag="wk")
            wpkT = cpool.tile([128, NCH, H], BF16, tag="wpkT")
            wpqT = cpool.tile([128, NCH, H], BF16, tag="wpqT")
            bqT = cpool.tile([128, NCH], F32, tag="bqT")
            bkT = cpool.tile([128, NCH], F32, tag="bkT")
            bpqd = cpool.tile([128, NCH, NH], BF16, tag="bpqd")
            ident = cpool.tile([128, 128], BF16, tag="ident")
            nc.sync.dma_start(hsTo[:], hsTo_d[:])
            nc.sync.dma_start(wq[:], wq_d[:])
            nc.sync.dma_start(wk[:], wk_d[:])
            nc.sync.dma_start(wpkT[:], wpkT_d[:])
            nc.sync.dma_start(wpqT[:], wpqT_d[:])
            nc.sync.dma_start(bqT[:], bqT_d[:])
            nc.sync.dma_start(bkT[:], bkT_d[:])
            nc.sync.dma_start(bpqd[:], bpqd_d[:])
            nc.sync.dma_start(ident[:], ident_d[:])
            bvbc = cpool.tile([128, H], BF16, tag="bvbc")
            nc.gpsimd.dma_start(bvbc[:], bv_d[:].partition_broadcast(128))
            mask12 = cpool.tile([NH, S], F32, tag="mask12")
            nc.gpsimd.dma_start(mask12[:], mask_d[:].partition_broadcast(NH))

            # ---- own-row projections qT_own / kT_own ----
            qTo = wpool.tile([128, NCH, TB], BF16, tag="qTo")
            kTo = wpool.tile([128, NCH, TB], BF16, tag="kTo")
            for m in range(NCH):
                pso = pspool.tile([128, TB], F32, tag="ps")
                for c in range(NCH):
                    nc.tensor.matmul(
                        pso[:], wq[:, c, m * 128 : (m + 1) * 128], hsTo[:, c, :],
                        start=(c == 0), stop=(c == NCH - 1),
                    )
                nc.vector.tensor_scalar_add(qTo[:, m, :], pso[:], bqT[:, m : m + 1])
                psk = pspool.tile([128, TB], F32, tag="ps")
                for c in range(NCH):
                    nc.tensor.matmul(
                        psk[:], wk[:, c, m * 128 : (m + 1) * 128], hsTo[:, c, :],
                        start=(c == 0), stop=(c == NCH - 1),
                    )
                nc.vector.tensor_scalar_add(kTo[:, m, :], psk[:], bkT[:, m : m + 1])

            # ---- qkp[c_chunk][128, t, 24]: cols 0:12 qpk (Wpk.T q), 12:24 kpq --
            # grouped 4 heads per PSUM tile -> one cast per group
            qkp = [
                wpool.tile([128, TB, 2 * NH], BF16, tag=f"qkp{m}", name=f"qkp{m}")
                for m in range(NCH)
            ]
            for m in range(NCH):
                for h in range(NH):
                    mh, oh = h // 2, (h % 2) * 64
                    ps1 = pspool.tile([128, TB], F32, tag="ps")
                    nc.tensor.matmul(
                        ps1[:],
                        wpkT[oh : oh + 64, mh, m * 128 : (m + 1) * 128],
                        qTo[oh : oh + 64, mh, :],
                    )
                    nc.scalar.activation(qkp[m][:, :, h], ps1[:], AF.Copy)
                    ps2 = pspool.tile([128, TB], F32, tag="ps")
                    nc.tensor.matmul(
                        ps2[:],
                        wpqT[oh : oh + 64, mh, m * 128 : (m + 1) * 128],
                        kTo[oh : oh + 64, mh, :],
                    )
                    nc.vector.tensor_copy(qkp[m][:, :, NH + h], ps2[:])

            # ---- main loop over own slabs t: c2p rows + p2c cols ----
            c2p_dram = dpool.tile([TB, NH, S], BF16)
            a2a_in = [dpool.tile([NC, NH, HT, TB], BF16, name=f"a2ai{u}") for u in range(2)]
            a2a_out = [dpool.tile([NC, NH, HT, TB], BF16, name=f"a2ao{u}") for u in range(2)]
            for t in range(TB):
                    if t % 2 == 0:
                        posT = ppool.tile([128, 2, NCH, S], BF16, tag="posT", name="posT")
                        nc.sync.dma_start(posT[:], pos_d[t // 2])
                    ps = pspool.tile([2 * NH, S], F32, tag="ps")
                    for m in range(NCH):
                        nc.tensor.matmul(
                            ps[:], qkp[m][:, t, :], posT[:, t % 2, m, :],
                            start=(m == 0), stop=(m == NCH - 1),
                        )
                    stage = spool.tile([2 * NH, S], BF16, tag="stage", name="stage")
                    nc.vector.tensor_copy(stage[:], ps[:])
                    nc.gpsimd.dma_start(c2p_dram[t], stage[0:NH, :])
                    nc.gpsimd.dma_start(
                        a2a_in[t // HT][:, :, t % HT, :].rearrange("d h i -> h d i"),
                        stage[NH : 2 * NH, :].rearrange("h (d i) -> h d i", d=NC),
                    )
                    if t == HT - 1 or t == TB - 1:
                        nc.gpsimd.collective_compute(
                            "AllToAll",
                            mybir.AluOpType.bypass,
                            replica_groups=[list(range(NC))],
                            ins=[a2a_in[t // HT].opt()],
                            outs=[a2a_out[t // HT].opt()],
                        )

            # ---- late constants (only needed after the loop) ----
            hsT = cpool.tile([128, NCH, S], BF16, tag="hsT")
            wv = cpool.tile([128, NCH, H], BF16, tag="wv")
            nc.sync.dma_start(hsT[:], hsT_d[:])
            nc.sync.dma_start(wv[:], wv_d[:])

            # ---- p2c rows half 0: transpose a2a_out [(s h t), il] -> [il, ...]
            p2c_rows = [
                wpool.tile([TB, NC * NH * HT], BF16, tag=f"p2c_rows{u}", name=f"p2c_rows{u}")
                for u in range(2)
            ]
            NT = NC * NH * HT // 128  # 18 tiles of 128 rows per half
            g2 = [
                wpool.tile([128, NT, TB], BF16, tag=f"g2_{u}", name=f"g2_{u}")
                for u in range(2)
            ]

            def transpose_half(u):
                nc.sync.dma_start(
                    g2[u][:],
                    a2a_out[u][:]
                    .rearrange("s h t i -> (s h t) i")
                    .rearrange("(m p) i -> p m i", p=128),
                )
                for m in range(NT):
                    pst = pspool.tile([TB, 128], BF16, tag="ps")
                    nc.tensor.transpose(pst[:], g2[u][:, m, :], ident[:])
                    nc.vector.tensor_copy(p2c_rows[u][:, m * 128 : (m + 1) * 128], pst[:])

            transpose_half(0)

            # ---- full kT, v, c2c, kb (overlaps the second AllToAll) ----
            kT = wpool.tile([128, NCH, S], BF16, tag="kT")
            for m in range(NCH):
                ps = pspool.tile([128, S], F32, tag="ps")
                for c in range(NCH):
                    nc.tensor.matmul(
                        ps[:], wk[:, c, m * 128 : (m + 1) * 128], hsT[:, c, :],
                        start=(c == 0), stop=(c == NCH - 1),
                    )
                nc.vector.tensor_scalar_add(kT[:, m, :], ps[:], bkT[:, m : m + 1])

            v_sb = wpool.tile([128, 3, H], BF16, tag="v_sb")
            for jc in range(3):
                for nh in range(2):
                    ps = pspool.tile([128, S], F32, tag="ps")
                    for c in range(NCH):
                        nc.tensor.matmul(
                            ps[:],
                            hsT[:, c, jc * 128 : (jc + 1) * 128],
                            wv[:, c, nh * S : (nh + 1) * S],
                            start=(c == 0), stop=(c == NCH - 1),
                        )
                    nc.scalar.activation(v_sb[:, jc, nh * S : (nh + 1) * S], ps[:], AF.Copy)
                nc.vector.tensor_tensor(v_sb[:, jc, :], v_sb[:, jc, :], bvbc[:], op=ADD)

            # colbias[h, j] = k[j].bpq_h + sqrt(D)*mask[j]; 1/sqrt(D) folds into exp
            pskb = pspool.tile([NH, S], F32, tag="ps")
            for m in range(NCH):
                nc.tensor.matmul(
                    pskb[:], bpqd[:, m, :], kT[:, m, :],
                    start=(m == 0), stop=(m == NCH - 1),
                )
            colbias = wpool.tile([NH, S], F32, tag="colbias")
            nc.vector.tensor_tensor(colbias[:], pskb[:], mask12[:], op=ADD)
            cb_dram = dpool.tile([NH, S], F32)
            nc.gpsimd.dma_start(cb_dram[:], colbias[:])
            colbias_bc = wpool.tile([TB, NH, S], F32, tag="colbias_bc")
            nc.gpsimd.dma_start(colbias_bc[:], cb_dram[:].partition_broadcast(TB))

            # ---- c2c rows (+ colbias in the same pass) ----
            scores = wpool.tile([TB, NH, S], F32, tag="scores")
            for h in range(NH):
                mh, oh = h // 2, (h % 2) * 64
                ps = pspool.tile([TB, S], F32, tag="ps")
                nc.tensor.matmul(ps[:], qTo[oh : oh + 64, mh, :], kT[oh : oh + 64, mh, :])
                nc.vector.tensor_tensor(scores[:, h, :], ps[:], colbias_bc[:, h, :], op=ADD)

            transpose_half(1)

            # ---- assemble scores rows, softmax ----
            c2p_rows = wpool.tile([TB, NH, S], BF16, tag="c2p_rows")
            nc.sync.dma_start(c2p_rows[:], c2p_dram[:])
            nc.vector.tensor_tensor(scores[:], scores[:], c2p_rows[:], op=ADD)
            sc4a = scores[:].rearrange("i h (s t) -> i h s t", s=NC)[:, :, :, 0:HT]
            nc.vector.tensor_tensor(
                sc4a,
                sc4a,
                p2c_rows[0][:].rearrange("i (s h t) -> i h s t", s=NC, h=NH),
                op=ADD,
            )

            negmax = wpool.tile([TB, NH], F32, tag="negmax")
            sums = wpool.tile([TB, NH], F32, tag="sums")
            recip = wpool.tile([TB, NH], F32, tag="recip")
            probs = wpool.tile([TB, NH, S], BF16, tag="probs")
            ptile = wpool.tile([128, 3, NH, TB], BF16, tag="ptile")
            out_sb = wpool.tile([TB, H], F32, tag="out_sb")
            isqd = 1.0 / math.sqrt(D)
            HG = 4  # heads per pipeline group
            for g in range(NH // HG):
                hs_, he = g * HG, (g + 1) * HG
                sc4b = scores[:, hs_:he, :].rearrange("i h (s t) -> i h s t", s=NC)[
                    :, :, :, HT:TB
                ]
                nc.vector.tensor_tensor(
                    sc4b,
                    sc4b,
                    p2c_rows[1][:]
                    .rearrange("i (s h t) -> i h s t", s=NC, h=NH)[:, hs_:he],
                    op=ADD,
                )
                for h in range(hs_, he):
                    nc.vector.tensor_reduce(
                        out=negmax[:, h : h + 1], in_=scores[:, h, :],
                        op=mybir.AluOpType.max, axis=mybir.AxisListType.X, negate=True,
                    )
                nc.vector.tensor_scalar_mul(
                    negmax[:, hs_:he], negmax[:, hs_:he], isqd
                )
                for h in range(hs_, he):
                    nc.scalar.activation(
                        probs[:, h, :], scores[:, h, :], AF.Exp,
                        bias=negmax[:, h : h + 1],
                        scale=isqd,
                        accum_out=sums[:, h : h + 1],
                    )
                for h in range(hs_, he):
                    for jc in range(3):
                        pst = pspool.tile([128, TB], BF16, tag="ps")
                        nc.tensor.transpose(
                            pst[:], probs[:, h, jc * 128 : (jc + 1) * 128],
                            ident[0:TB, 0:TB],
                        )
                        nc.vector.tensor_copy(ptile[:, jc, h, :], pst[:])
                nc.vector.reciprocal(recip[:, hs_:he], sums[:, hs_:he])
                for h in range(hs_, he):
                    psc = pspool.tile([TB, D], F32, tag="ps")
                    for jc in range(3):
                        nc.tensor.matmul(
                            psc[:], ptile[:, jc, h, :], v_sb[:, jc, h * D : (h + 1) * D],
                            start=(jc == 0), stop=(jc == 2),
                        )
                    nc.scalar.activation(
                        out_sb[:, h * D : (h + 1) * D], psc[:], AF.Copy,
                        scale=recip[:, h : h + 1],
                    )
                nc.sync.dma_start(
                    out_d[:, hs_ * D : he * D], out_sb[:, hs_ * D : he * D]
                )

    nc.compile()
    return nc


_NC_CACHE = None


def _chunked(w):
    """[H, X] f32 -> [128, NCH, X] bf16 with [p, m, x] = w[128m+p, x]."""
    bf = ml_dtypes.bfloat16
    X = w.shape[1]
    return np.ascontiguousarray(
        np.asarray(w, np.float32).reshape(NCH, 128, X).transpose(1, 0, 2)
    ).astype(bf)


def _prep_inputs(hidden_states, attention_mask, pos_emb, Wq, bq, Wk, bk, Wv, bv,
                 Wpk, bpk, Wpq, bpq):
    bf = ml_dtypes.bfloat16
    hs = np.ascontiguousarray(np.asarray(hidden_states, np.float32)[0])  # (S, H)
    hsT = np.ascontiguousarray(hs.T)  # (H, S) f32
    bqT = np.ascontiguousarray(np.asarray(bq, np.float32).reshape(NCH, 128).T)
    bkT = np.ascontiguousarray(np.asarray(bk, np.float32).reshape(NCH, 128).T)
    bpq_f = np.asarray(bpq, np.float32)
    bpqd = np.zeros((128, NCH, NH), bf)
    for m in range(NCH):
        for half in range(2):
            h = 2 * m + half
            bpqd[64 * half : 64 * half + 64, m, h] = bpq_f[
                128 * m + 64 * half : 128 * m + 64 * half + 64
            ].astype(bf)
    mask_row = (
        np.ascontiguousarray(np.asarray(attention_mask, np.float32)[0, 0, 0])
        * math.sqrt(D)
    )
    ident = np.eye(128, dtype=bf)

    common = dict(
        hsT=_chunked(hsT),
        wq=_chunked(np.asarray(Wq)), wk=_chunked(np.asarray(Wk)),
        wv=_chunked(np.asarray(Wv)),
        wpkT=_chunked(np.ascontiguousarray(np.asarray(Wpk, np.float32).T)),
        wpqT=_chunked(np.ascontiguousarray(np.asarray(Wpq, np.float32).T)),
        bqT=bqT, bkT=bkT, bv=np.asarray(bv, np.float32),
        bpqd=bpqd, maskrow=mask_row, ident=ident,
    )
    in_maps = []
    pos0 = np.asarray(pos_emb)[0]  # (S, S, H) f32
    for c in range(NC):
        sl = slice(c * TB, (c + 1) * TB)
        m = dict(common)
        # [t, p, mm, x] = pos[t0+t, x, 128*mm+p]: one DMA per slab pair with
        # contiguous (NCH*S*2)B partition lines
        m["pos"] = (
            pos0[sl]
            .transpose(0, 2, 1)
            .reshape(TB // 2, 2, NCH, 128, S)
            .transpose(0, 3, 1, 2, 4)
            .astype(bf)
        )
        m["hsTo"] = _chunked(hsT[:, sl])
        in_maps.append(m)
    return in_maps


def kernel(**inputs):
    global _NC_CACHE
    if _NC_CACHE is None:
        _NC_CACHE = build_module()
    nc = _NC_CACHE
    in_maps = _prep_inputs(**inputs)
    res = run_bass_kernel_spmd(nc, in_maps, core_ids=list(range(NC)))
    out = np.concatenate([r["out"] for r in res.results], axis=0)
    return out.reshape(1, S, H).astype(np.float32)
